# revision 9
# baseline (speedup 1.0000x reference)
"""IterativeCorrelationLayer on 8 Trainium2 NeuronCores (Bass/Tile).

Contract: kernel(**inputs) takes FULL unsharded inputs
  left_feature  (8, 256, 64, 128) f32
  right_feature (8, 256, 64, 128) f32
  flow          (8, 2, 64, 128)   f32
and returns the FULL output (8, 36, 64, 128) f32:
  bilinear warp of right_feature by (grid + flow) with zeros padding,
  then a (1, 9) windowed group correlation (4 groups of 64 channels,
  replicate-padded x-shifts dx in [-4, 4]), mean over group channels.

Sharding: data-parallel over batch B=8, one sample per NeuronCore; no
cross-core communication.

Device algorithm (per core / sample):
  - host precomputes, from flow: per-pixel gather indices of the 4
    bilinear corner pixels (as 2-pixel-pair rows of a channel-last copy
    of right) and the 4 combined corner weights (zero-padding folded in).
  - dma_gather (transpose mode) pulls corner channel-vectors into SBUF
    with channels on partitions; DVE combines them into the warped image
    (bf16); DVE forms the 9 shifted left*warped products; TensorE
    contracts channels with per-(tile,shift) indicator matmuls
    accumulated into one (36, 512) PSUM tile per pixel block; ScalarE
    scales by 1/64 and a permuted-AP DMA writes straight to the output
    layout.
"""

import os
import sys
from concurrent.futures import ThreadPoolExecutor
from contextlib import ExitStack

import ml_dtypes
import numpy as np

B = 8
H, W, C = 64, 128, 256
HW = H * W            # 8192
PADL = 4
PADW = W + 16         # 144 (4 left pad; >=4 right pad, rest unused)
PHW = H * PADW        # 9216
GROUPS = 4
WX = 9

NCHUNK = 4            # chunks over H
CH = H // NCHUNK      # 16 rows per chunk
NPIX = CH * PADW      # 2304 padded pixels / chunk
NOUT = CH * W         # 2048 output pixels / chunk
NBLK = NOUT // 512    # 4 psum blocks of 512 pixels
NSUB = 3              # sub-gathers per chunk (SWDGE desc-ring limit)
SUB = NPIX // NSUB    # 768 idxs per sub-gather

_BF16 = ml_dtypes.bfloat16

_PW_ARANGE = np.arange(PADW)
_WSRC = np.clip(_PW_ARANGE - PADL, 0, W - 1)
_H_COL = np.arange(H, dtype=np.float32)[:, None]


def _prep_one(args):
    """Per-sample host prep: cast/transpose + gather indices + weights."""
    left, right, flow = args
    left_bf = np.ascontiguousarray(left.reshape(C, HW)).astype(_BF16)

    rcl = np.empty((HW + 2, C), dtype=_BF16)
    rcl[:HW] = right.reshape(C, HW).T.astype(_BF16)
    rcl[HW:] = 0

    fx, fy = flow[0], flow[1]                      # (H, W)
    xs = _WSRC[None, :] + fx[:, _WSRC]             # (H, PADW)
    ys = _H_COL + fy[:, _WSRC]
    x0 = np.floor(xs)
    y0 = np.floor(ys)
    wx1 = xs - x0
    wy1 = ys - y0

    # slot weights: the gathered pair covers absolute x = xc, xc+1
    wxs0 = np.where((x0 >= 0) & (x0 <= W - 1), 1.0 - wx1,
                    np.where(x0 == -1, wx1, 0.0))
    wxs1 = np.where((x0 >= 0) & (x0 + 1 <= W - 1), wx1, 0.0)
    wys0 = np.where((y0 >= 0) & (y0 <= H - 1), 1.0 - wy1,
                    np.where(y0 == -1, wy1, 0.0))
    wys1 = np.where((y0 >= 0) & (y0 + 1 <= H - 1), wy1, 0.0)

    wts = np.empty((4, PHW), dtype=_BF16)
    wts[0] = (wys0 * wxs0).ravel()
    wts[1] = (wys0 * wxs1).ravel()
    wts[2] = (wys1 * wxs0).ravel()
    wts[3] = (wys1 * wxs1).ravel()

    xc = np.clip(x0, 0, W - 1).astype(np.int32)
    yc0 = np.clip(y0, 0, H - 1).astype(np.int32)
    yc1 = np.clip(y0 + 1, 0, H - 1).astype(np.int32)
    idx0 = (yc0 * W + xc).ravel()
    idx1 = (yc1 * W + xc).ravel()

    def wrap16(idx):
        w = idx.reshape(PHW // 16, 16).T.astype(np.int16)   # (16, PHW//16)
        return np.ascontiguousarray(np.tile(w, (8, 1)))     # (128, PHW//16)

    return {
        "left": left_bf,
        "rcl": rcl,
        "idx0": wrap16(idx0),
        "idx1": wrap16(idx1),
        "wts": wts,
    }


def _eye128():
    e = np.zeros((128, 128), dtype=_BF16)
    np.fill_diagonal(e, 1.0)
    return e


def _indicators():
    # per-(t,k) stationary: (128, 18*36), column block j = 9t+k,
    # ind_j[c, m] = 1 iff m == 4k + 2t + (c >= 64)
    inds = np.zeros((128, 18 * 36), dtype=_BF16)
    for t in range(2):
        for k in range(WX):
            j = 9 * t + k
            inds[:64, 36 * j + 4 * k + 2 * t] = 1.0
            inds[64:, 36 * j + 4 * k + 2 * t + 1] = 1.0
    return inds


def _build_kernel(ctx, tc, nc, reps=1):
    import concourse.tile as tile  # noqa: F401
    from concourse import library_config, mybir
    from concourse.ap import AP as RawAP

    bf = mybir.dt.bfloat16
    left_d = nc.dram_tensor("left", [C, HW], bf, kind="ExternalInput").ap()
    rcl_d = nc.dram_tensor("rcl", [HW + 2, C], bf, kind="ExternalInput").ap()
    idx0_d = nc.dram_tensor("idx0", [128, PHW // 16], mybir.dt.int16,
                            kind="ExternalInput").ap()
    idx1_d = nc.dram_tensor("idx1", [128, PHW // 16], mybir.dt.int16,
                            kind="ExternalInput").ap()
    wts_d = nc.dram_tensor("wts", [4, PHW], bf, kind="ExternalInput").ap()
    ind_d = nc.dram_tensor("ind", [128, 18 * 36], bf,
                           kind="ExternalInput").ap()
    eye_d = nc.dram_tensor("eye", [128, 128], bf, kind="ExternalInput").ap()
    out_d = nc.dram_tensor("out", [GROUPS * WX, HW], mybir.dt.float32,
                           kind="ExternalOutput").ap()

    nc.gpsimd.load_library(library_config.mlp)

    const_pool = ctx.enter_context(tc.tile_pool(name="const", bufs=1))
    io_pool = ctx.enter_context(tc.tile_pool(name="io", bufs=2))
    wk_pool = ctx.enter_context(tc.tile_pool(name="wk", bufs=1))
    prod_pool = ctx.enter_context(tc.tile_pool(name="prod", bufs=3))
    ps_pool = ctx.enter_context(tc.tile_pool(name="ps", bufs=1, space="PSUM"))
    wps_pool = ctx.enter_context(tc.tile_pool(name="wps", bufs=3, space="PSUM"))
    os_pool = ctx.enter_context(tc.tile_pool(name="os", bufs=1))

    ind = const_pool.tile([128, 18 * 36], bf)
    nc.sync.dma_start(ind[:], ind_d[:])
    eye = const_pool.tile([128, 128], bf)
    nc.sync.dma_start(eye[:], eye_d[:])

    # gather source view: rows of 512 bf16 (2 adjacent pixels), stride 256
    src = RawAP(tensor=rcl_d.tensor, offset=0, ap=[[256, HW + 1], [1, 512]])

    out_r = out_d.rearrange("(g k) x -> k g x", g=GROUPS)

    for rep_c in range(reps * NCHUNK):
        rep, c = divmod(rep_c, NCHUNK)
        qpix = c * NPIX
        opix = c * NOUT

        idx0_t = io_pool.tile([128, NPIX // 16], mybir.dt.int16, tag="idx0",
                              name=f"idx0_{rep_c}")
        idx1_t = io_pool.tile([128, NPIX // 16], mybir.dt.int16, tag="idx1",
                              name=f"idx1_{rep_c}")
        nc.sync.dma_start(idx0_t[:], idx0_d[:, qpix // 16:(qpix + NPIX) // 16])
        nc.sync.dma_start(idx1_t[:], idx1_d[:, qpix // 16:(qpix + NPIX) // 16])

        g0 = io_pool.tile([128, NSUB, 4, SUB], bf, tag="g0", name=f"g0_{rep_c}")
        g1 = io_pool.tile([128, NSUB, 4, SUB], bf, tag="g1", name=f"g1_{rep_c}")
        for s in range(NSUB):
            cols = slice(s * SUB // 16, (s + 1) * SUB // 16)
            nc.gpsimd.dma_gather(g0[:, s, :, :], src, idx0_t[:, cols],
                                 SUB, SUB, 512, elem_step=256, transpose=True)
            nc.gpsimd.dma_gather(g1[:, s, :, :], src, idx1_t[:, cols],
                                 SUB, SUB, 512, elem_step=256, transpose=True)

        # corner weights replicated to all partitions: (128, 4, NPIX)
        w4 = io_pool.tile([128, 4, NPIX], bf, tag="w4", name=f"w4_{rep_c}")
        nc.sync.dma_start(
            w4[:], wts_d[:, qpix:qpix + NPIX].partition_broadcast(128))

        lts = []
        for t in range(2):
            lt = io_pool.tile([128, NOUT], bf, tag=f"l{t}", name=f"l{t}_{rep_c}")
            nc.sync.dma_start(
                lt[:], left_d[128 * t:128 * (t + 1), opix:opix + NOUT])
            lts.append(lt)

        psts = [ps_pool.tile([36, 512], mybir.dt.float32, tag=f"ps{b}",
                             name=f"ps{b}_{rep_c}")
                for b in range(NBLK)]

        for t in range(2):
            # corner products on DVE; adds via PE identity-accumulate in PSUM
            cps = []
            for a in range(4):
                cp = wk_pool.tile([128, NPIX], bf, tag=f"cp{t}{a}",
                                  name=f"cp{t}{a}_{rep_c}")
                wv = w4[:, a, :].rearrange("p (s i) -> p s i", i=SUB)
                gsrc = (g0 if a < 2 else g1)[:, :, t + 2 * (a % 2), :]
                nc.vector.tensor_mul(
                    cp[:].rearrange("p (s i) -> p s i", i=SUB), wv, gsrc)
                cps.append(cp)
            wp = wk_pool.tile([128, NPIX], bf, tag=f"wp{t}", name=f"wp{t}_{rep_c}")
            wpo = wk_pool.tile([128, NPIX], bf, tag=f"wpo{t}",
                               name=f"wpo{t}_{rep_c}")
            npc = (NPIX + 511) // 512
            for pz in range(npc):
                lo = 512 * pz
                n = min(512, NPIX - lo)
                wps = wps_pool.tile([128, 512], mybir.dt.float32, tag="wps",
                                    name=f"wps{t}{pz}_{rep_c}")
                for a in range(4):
                    nc.tensor.matmul(wps[:, 0:n], eye[:],
                                     cps[a][:, lo:lo + n],
                                     start=(a == 0), stop=(a == 3))
                nc.scalar.copy(wp[:, lo:lo + n], wps[:, 0:n])
                if pz == 0:
                    nc.scalar.copy(wpo[:, 0:n - 1], wps[:, 1:n])
                else:
                    nc.scalar.copy(wpo[:, lo - 1:lo + n - 1], wps[:, 0:n])

            lv = lts[t][:].rearrange("p (h w) -> p h w", w=W)
            for k in range(WX):
                if k % 2 == 0:
                    wv = wp[:].rearrange("p (h w) -> p h w", w=PADW)[
                        :, :, k:k + W]
                else:
                    wv = wpo[:].rearrange("p (h w) -> p h w", w=PADW)[
                        :, :, k - 1:k - 1 + W]
                prod = prod_pool.tile([128, NOUT], bf, tag="prod",
                                      name=f"prod{t}_{k}_{rep_c}")
                nc.vector.tensor_mul(
                    prod[:].rearrange("p (h w) -> p h w", w=W), lv, wv)
                j = 9 * t + k
                for b in range(NBLK):
                    nc.tensor.matmul(
                        psts[b][:, :],
                        ind[:, 36 * j:36 * (j + 1)],
                        prod[:, 512 * b:512 * (b + 1)],
                        start=(t == 0 and k == 0),
                        stop=(t == 1 and k == WX - 1))

        for b in range(NBLK):
            osb = os_pool.tile([36, 512], mybir.dt.float32, tag=f"os{b}",
                               name=f"os{b}_{rep_c}")
            nc.scalar.mul(osb[:], psts[b][:], 1.0 / 64.0)
            pix0 = opix + 512 * b
            nc.sync.dma_start(out_r[:, :, pix0:pix0 + 512], osb[:])


_STATE = None


def _make_exec(nc):
    """Build a cached 8-core jitted dispatcher for a compiled Bacc module."""
    import jax
    import numpy as _np
    import jax.numpy as jnp
    from jax.sharding import Mesh, PartitionSpec
    from jax.experimental.shard_map import shard_map

    from concourse.bass2jax import (
        _bass_exec_p, install_neuronx_cc_hook, partition_id_tensor)
    import concourse.mybir as mybir

    install_neuronx_cc_hook()

    in_names = []
    out_names = []
    out_avals = []
    partition_name = (nc.partition_id_tensor.name
                      if nc.partition_id_tensor else None)
    for alloc in nc.m.functions[0].allocations:
        if not isinstance(alloc, mybir.MemoryLocationSet):
            continue
        name = alloc.memorylocations[0].name
        if alloc.kind == "ExternalInput":
            if name != partition_name:
                in_names.append(name)
        elif alloc.kind == "ExternalOutput":
            out_names.append(name)
            shape = tuple(alloc.tensor_shape)
            dtype = mybir.dt.np(alloc.dtype)
            out_avals.append(jax.core.ShapedArray(shape, dtype))
    n_params = len(in_names)
    all_names = list(in_names) + list(out_names)
    if partition_name is not None:
        all_names.append(partition_name)

    def _body(*args):
        operands = list(args)
        if partition_name is not None:
            operands.append(partition_id_tensor())
        outs = _bass_exec_p.bind(
            *operands,
            out_avals=tuple(out_avals),
            in_names=tuple(all_names),
            out_names=tuple(out_names),
            lowering_input_output_aliases=(),
            sim_require_finite=True,
            sim_require_nnan=True,
            nc=nc,
        )
        return tuple(outs)

    n_outs = len(out_names)
    donate = tuple(range(n_params, n_params + n_outs))
    devices = jax.devices("axon")[:B]
    mesh = Mesh(np.asarray(devices), ("core",))
    in_specs = (PartitionSpec("core"),) * (n_params + n_outs)
    out_specs = (PartitionSpec("core"),) * n_outs
    sharded = jax.jit(
        shard_map(_body, mesh=mesh, in_specs=in_specs, out_specs=out_specs,
                  check_rep=False),
        donate_argnums=donate, keep_unused=True)

    # on-device zero output buffers (no host->device transfer)
    from jax.sharding import NamedSharding
    zsh = NamedSharding(mesh, PartitionSpec("core"))
    zshapes = [(B * a.shape[0],) + tuple(a.shape[1:]) for a in out_avals]
    zdtypes = [a.dtype for a in out_avals]

    def make_zeros():
        return [jax.jit(lambda s=s, d=d: jnp.zeros(s, d),
                        out_shardings=zsh)()
                for s, d in zip(zshapes, zdtypes)]

    return {"sharded": sharded, "in_names": in_names,
            "out_names": out_names, "make_zeros": make_zeros}


def _build_module(reps=1):
    import concourse.tile as tile
    from concourse import bacc

    nc = bacc.Bacc("TRN2", target_bir_lowering=False, debug=False,
                   enable_asserts=True)
    with tile.TileContext(nc, trace_sim=False) as tc:
        with ExitStack() as ctx:
            _build_kernel(ctx, tc, nc, reps=reps)
    nc.compile()
    return nc


def _ensure_built():
    global _STATE
    if _STATE is not None:
        return _STATE
    os.environ.setdefault("JAX_PLATFORMS", "axon")
    _STATE = _make_exec(_build_module())
    return _STATE


_IND_CONCAT = None


def _host_prep_all(left_feature, right_feature, flow):
    """Concatenated per-core device inputs, keyed by tensor name."""
    global _IND_CONCAT
    args = [(left_feature[b], right_feature[b], flow[b]) for b in range(B)]
    with ThreadPoolExecutor(max_workers=B) as ex:
        per = list(ex.map(_prep_one, args))
    if _IND_CONCAT is None:
        _IND_CONCAT = (np.concatenate([_indicators()] * B, axis=0),
                       np.concatenate([_eye128()] * B, axis=0))
    concat = {
        name: np.concatenate([p[name] for p in per], axis=0)
        for name in ("left", "rcl", "idx0", "idx1", "wts")
    }
    concat["ind"] = _IND_CONCAT[0]
    concat["eye"] = _IND_CONCAT[1]
    return concat


def _run_device(concat):
    """Dispatch the compiled 8-core NEFF; returns (8*36, 8192) f32."""
    st = _ensure_built()
    ins = [concat[name] for name in st["in_names"]]
    out_arrs = st["sharded"](*ins, *st["make_zeros"]())
    return np.asarray(out_arrs[0])


def kernel(left_feature, right_feature, flow):
    left_feature = np.ascontiguousarray(left_feature, dtype=np.float32)
    right_feature = np.ascontiguousarray(right_feature, dtype=np.float32)
    flow = np.ascontiguousarray(flow, dtype=np.float32)
    concat = _host_prep_all(left_feature, right_feature, flow)
    flat = _run_device(concat)
    return flat.reshape(B, GROUPS * WX, H, W)


def warmup():
    """Compile + one dummy run so later kernel() calls are warm."""
    z = np.zeros((B, C, H, W), np.float32)
    zf = np.zeros((B, 2, H, W), np.float32)
    kernel(z, z, zf)


try:
    warmup()
except Exception:
    pass
